# revision 5
# baseline (speedup 1.0000x reference)
"""Multi-head attention (B=2, S=2048, D=1024, H=16) on 8 trn2 NeuronCores.

Sharding: batch (2) x head-groups (4 heads each, 4 groups) = 8 cores.
Each core computes Q/K/V projections for its 4 heads on its batch,
causal-masked softmax attention, and a partial output projection
(row-sharded w_o); the host sums the 4 partials per batch.

Layout strategy: the host stages transposed inputs (xT = x[b].T) so every
matmul contraction runs over the SBUF partition axis with no on-device
transposes. Attention scores are computed transposed (ST[k, q]) so that
P = exp(ST) serves directly as the PV matmul's moving operand, and V with
an appended ones-column yields the softmax denominator for free.
The input projections run in bf16; everything downstream runs float32r
(full-rate PE, fp32 storage).
"""
import sys

sys.path.insert(0, "/opt/trn_rl_repo")

import numpy as np
import ml_dtypes

import concourse.bass as bass
import concourse.mybir as mybir
import concourse.tile as tile
from concourse.bass_utils import run_bass_kernel_spmd

B, S, D, H, DK = 2, 2048, 1024, 16, 64
NCORES = 8
HG = 4                # heads per core
DHG = HG * DK         # 256 head-dims per core
KT = D // 128         # 8 contraction tiles for the projections
ST128 = S // 128      # 16 128-row tiles of S
QS = 512              # q-strip width
NQS = S // QS         # 4 strips
NEG = np.float32(-1e9)

f32 = mybir.dt.float32
f32r = mybir.dt.float32r
bf16 = mybir.dt.bfloat16
EXP = mybir.ActivationFunctionType.Exp


def _split_waits(nc, max_waits=1):
    """This walrus build rejects >1 SyncWait per instruction (and >0 on
    fp32-family matmuls, which lower through the 1-wait S3_LW struct).
    Hoist excess waits onto dedicated NOPs on the same engine queue."""
    n = 0
    for fn in nc.m.functions:
        for blk in fn.blocks:
            new = []
            for ins in blk.instructions:
                si = getattr(ins, "sync_info", None)
                if si is not None and si.on_wait:
                    limit = 0 if isinstance(ins, mybir.InstMatmult) else max_waits
                    if len(si.on_wait) > limit:
                        waits = list(si.on_wait)
                        hoist = waits if limit == 0 else waits[:-limit]
                        keep = [] if limit == 0 else waits[-limit:]
                        for w in hoist:
                            n += 1
                            new.append(
                                mybir.InstNoOp(
                                    name=f"I-waitfix-{n}",
                                    engine=ins.engine,
                                    bass_nofuse=True,
                                    sync_info=mybir.SyncInfo(
                                        on_wait=[w], on_update=[]
                                    ),
                                )
                            )
                        ins.sync_info = mybir.SyncInfo(
                            on_wait=keep, on_update=list(si.on_update)
                        )
                new.append(ins)
            blk.instructions[:] = new
    return n


def classify_mask(maskT):
    """Block-classify the transposed mask at 128x128 granularity.
    Returns (cls[i,j] in {0 empty,1 full,2 partial}, bias index map,
    list of additive fp32 bias blocks for the partial ones)."""
    nb = S // 128
    cls = np.empty((nb, nb), dtype=np.int8)
    bidx = np.full((nb, nb), -1, dtype=np.int32)
    biases = []
    for i in range(nb):
        for j in range(nb):
            blk = maskT[i * 128 : (i + 1) * 128, j * 128 : (j + 1) * 128]
            if (blk != 0).all():
                cls[i, j] = 1
            elif (blk == 0).all():
                cls[i, j] = 0
            else:
                cls[i, j] = 2
                bidx[i, j] = len(biases)
                biases.append(
                    np.where(blk == 0, NEG, np.float32(0.0)).astype(np.float32)
                )
    return cls, bidx, biases


def build_program(cls, bidx, n_bias):
    nb_alloc = max(1, n_bias)
    nc = bass.Bass("TRN2", target_bir_lowering=False, debug=False,
                   num_devices=NCORES)
    xq_d = nc.dram_tensor("xqT", [D, S], bf16, kind="ExternalInput").ap()
    xk_d = nc.dram_tensor("xkT", [D, S], bf16, kind="ExternalInput").ap()
    xv_d = nc.dram_tensor("xvT", [D, S], bf16, kind="ExternalInput").ap()
    wq_d = nc.dram_tensor("wqT", [D, DHG], bf16, kind="ExternalInput").ap()
    wk_d = nc.dram_tensor("wkT", [D, DHG], bf16, kind="ExternalInput").ap()
    wv_d = nc.dram_tensor("wvT", [D, DHG], bf16, kind="ExternalInput").ap()
    wo_d = nc.dram_tensor("woT", [DHG, D], f32r, kind="ExternalInput").ap()
    bias_d = nc.dram_tensor("biasT", [nb_alloc, 128, 128], f32,
                            kind="ExternalInput").ap()
    ones_d = nc.dram_tensor("onesr", [1, DK], f32r, kind="ExternalInput").ap()
    y_d = nc.dram_tensor("y", [S, D], f32, kind="ExternalOutput").ap()

    with tile.TileContext(nc) as tc:
        with tc.tile_pool(name="persist", bufs=1) as pp:
            qt_sb = pp.tile([128, 2, S], f32r)            # Q^T, 2 head-pairs
            kt_sb = pp.tile([128, 2, S], f32r)            # K^T
            v_sb = pp.tile([128, ST128, HG, DK + 1], bf16)  # V + ones col
            ot_sb = pp.tile([128, 2, S], f32r)            # attention out^T
            wo_sb = pp.tile([128, 2, D], f32r)
            bias_sb = pp.tile([128, nb_alloc, 128], f32)
            ones_row = pp.tile([1, DK], f32r)
            nc.sync.dma_start(out=ones_row[:], in_=ones_d[:])
            # ones column: memset ALL of v_sb; the V-projection copies
            # overwrite cols 0..DK-1, leaving col DK at 1.0
            nc.vector.memset(
                v_sb[:].rearrange("p a b c -> p (a b c)"), 1.0
            )
            nc.sync.dma_start(
                out=wo_sb[:], in_=wo_d.rearrange("(n p) d -> p n d", p=128)
            )
            if n_bias:
                nc.sync.dma_start(
                    out=bias_sb[:], in_=bias_d.rearrange("n p c -> p n c")
                )

            # ---- Phase A: projections (bf16) ----
            with tc.tile_pool(name="xw", bufs=2) as xw, tc.tile_pool(
                name="psA", bufs=2, space="PSUM"
            ) as psA:
                for x_d, w_d, which in (
                    (xq_d, wq_d, "q"),
                    (xk_d, wk_d, "k"),
                    (xv_d, wv_d, "v"),
                ):
                    xt = xw.tile([128, KT, S], bf16, tag="xT")
                    wt = xw.tile([128, KT, DHG], bf16, tag="wT")
                    nc.sync.dma_start(
                        out=xt[:], in_=x_d.rearrange("(n p) s -> p n s", p=128)
                    )
                    nc.sync.dma_start(
                        out=wt[:], in_=w_d.rearrange("(n p) s -> p n s", p=128)
                    )
                    if which in ("q", "k"):
                        dst = qt_sb if which == "q" else kt_sb
                        for mt in range(2):
                            for qs in range(NQS):
                                ps = psA.tile([128, QS], f32, tag="pA")
                                for kt in range(KT):
                                    nc.tensor.matmul(
                                        ps[:],
                                        wt[:, kt, mt * 128 : (mt + 1) * 128],
                                        xt[:, kt, qs * QS : (qs + 1) * QS],
                                        start=(kt == 0),
                                        stop=(kt == KT - 1),
                                    )
                                nc.vector.tensor_copy(
                                    out=dst[:, mt, qs * QS : (qs + 1) * QS],
                                    in_=ps[:],
                                )
                    else:
                        for st in range(ST128):
                            ps = psA.tile([128, QS], f32, tag="pA")
                            for kt in range(KT):
                                nc.tensor.matmul(
                                    ps[:, :DHG],
                                    xt[:, kt, st * 128 : (st + 1) * 128],
                                    wt[:, kt, :],
                                    start=(kt == 0),
                                    stop=(kt == KT - 1),
                                )
                            nc.vector.tensor_copy(
                                out=v_sb[:, st, :, 0:DK],
                                in_=ps[:, :DHG].rearrange(
                                    "p (h d) -> p h d", h=HG
                                ),
                            )

            # ---- Phase B: attention ----
            with tc.tile_pool(name="pb", bufs=4) as pb, tc.tile_pool(
                name="bc", bufs=2
            ) as bcp, tc.tile_pool(
                name="psS", bufs=2, space="PSUM"
            ) as psS, tc.tile_pool(
                name="psOT", bufs=2, space="PSUM"
            ) as psOT, tc.tile_pool(
                name="psBC", bufs=2, space="PSUM"
            ) as psBC:
                for h in range(HG):
                    po = 64 * (h % 2)
                    mt = h // 2
                    for qs in range(NQS):
                        sub_all = cls[:, 4 * qs : 4 * qs + 4]
                        kts = [i for i in range(ST128) if sub_all[i].any()]
                        pot = psOT.tile([DK + 1, QS], f32, tag="pot")
                        for idx, kt in enumerate(kts):
                            sub = sub_all[kt]
                            nz = np.nonzero(sub)[0]
                            c0 = int(nz.min()) * 128
                            c1 = (int(nz.max()) + 1) * 128
                            ps = psS.tile([128, QS], f32, tag="ps")
                            nc.tensor.matmul(
                                ps[:, c0:c1],
                                kt_sb[
                                    po : po + 64, mt,
                                    kt * 128 : (kt + 1) * 128,
                                ],
                                qt_sb[
                                    po : po + 64, mt,
                                    qs * QS + c0 : qs * QS + c1,
                                ],
                                start=True,
                                stop=True,
                            )
                            for j in range(4):
                                if sub[j] == 2:
                                    bi = int(bidx[kt, 4 * qs + j])
                                    nc.vector.tensor_add(
                                        ps[:, j * 128 : (j + 1) * 128],
                                        ps[:, j * 128 : (j + 1) * 128],
                                        bias_sb[:, bi, :],
                                    )
                            p_sb = pb.tile([128, QS], bf16, tag="p")
                            if c0 > 0:
                                nc.vector.memset(p_sb[:, 0:c0], 0.0)
                            if c1 < QS:
                                nc.vector.memset(p_sb[:, c1:], 0.0)
                            nc.scalar.activation(
                                p_sb[:, c0:c1], ps[:, c0:c1], EXP, scale=0.125
                            )
                            nc.tensor.matmul(
                                pot[:],
                                v_sb[:, kt, h, :],
                                p_sb[:],
                                start=(idx == 0),
                                stop=(idx == len(kts) - 1),
                            )
                        recip = bcp.tile([1, QS], f32r, tag="recip")
                        with nc.allow_low_precision(
                            reason="f32r reciprocal feeds f32r matmul"
                        ):
                            nc.vector.reciprocal(
                                recip[:], pot[DK : DK + 1, :]
                            )
                        pbc = psBC.tile([DK, QS], f32, tag="pbc")
                        nc.tensor.matmul(
                            pbc[:],
                            ones_row[:],
                            recip[:],
                            start=True,
                            stop=True,
                        )
                        bcs = bcp.tile([DK, QS], f32, tag="bc")
                        nc.vector.tensor_copy(out=bcs[:], in_=pbc[:])
                        nc.vector.tensor_mul(
                            ot_sb[po : po + 64, mt, qs * QS : (qs + 1) * QS],
                            pot[0:DK, :],
                            bcs[:],
                        )

            # ---- Phase C: output projection (partial w_o) ----
            with tc.tile_pool(name="yp", bufs=3) as yp, tc.tile_pool(
                name="psC", bufs=2, space="PSUM"
            ) as psC:
                for st in range(ST128):
                    for nh in range(2):
                        ps = psC.tile([128, QS], f32, tag="py")
                        for mt in range(2):
                            nc.tensor.matmul(
                                ps[:],
                                ot_sb[
                                    :, mt, st * 128 : (st + 1) * 128
                                ],
                                wo_sb[:, mt, nh * QS : (nh + 1) * QS].bitcast(
                                    f32r
                                ),
                                start=(mt == 0),
                                stop=(mt == 1),
                            )
                        y_sb = yp.tile([128, QS], f32, tag="y")
                        nc.vector.tensor_copy(out=y_sb[:], in_=ps[:])
                        nc.sync.dma_start(
                            out=y_d[
                                st * 128 : (st + 1) * 128,
                                nh * QS : (nh + 1) * QS,
                            ],
                            in_=y_sb[:],
                        )

    _split_waits(nc)
    return nc


_program_cache = {}


def get_program(cls, bidx, n_bias):
    key = (cls.tobytes(), bidx.tobytes(), n_bias)
    if key not in _program_cache:
        _program_cache[key] = build_program(cls, bidx, n_bias)
    return _program_cache[key]


def make_in_maps(q, k, v, mask, w_q, w_k, w_v, w_o, biases):
    bias_arr = (
        np.stack(biases).astype(np.float32)
        if biases
        else np.zeros((1, 128, 128), np.float32)
    )
    in_maps = []
    for c in range(NCORES):
        b, g = divmod(c, 4)
        rows = slice(g * DHG, (g + 1) * DHG)
        in_maps.append(
            {
                "xqT": np.ascontiguousarray(q[b].T).astype(ml_dtypes.bfloat16),
                "xkT": np.ascontiguousarray(k[b].T).astype(ml_dtypes.bfloat16),
                "xvT": np.ascontiguousarray(v[b].T).astype(ml_dtypes.bfloat16),
                "wqT": np.ascontiguousarray(w_q[rows].T).astype(
                    ml_dtypes.bfloat16
                ),
                "wkT": np.ascontiguousarray(w_k[rows].T).astype(
                    ml_dtypes.bfloat16
                ),
                "wvT": np.ascontiguousarray(w_v[rows].T).astype(
                    ml_dtypes.bfloat16
                ),
                "woT": np.ascontiguousarray(w_o[:, rows].T).astype(np.float32),
                "biasT": bias_arr,
                "onesr": np.ones((1, DK), np.float32),
            }
        )
    return in_maps


def combine_results(results):
    out = np.empty((B, S, D), np.float32)
    for b in range(B):
        acc = results[4 * b]["y"].astype(np.float32).copy()
        for g in range(1, 4):
            acc += results[4 * b + g]["y"]
        out[b] = acc
    return out


def kernel(q, k, v, mask, w_q, w_k, w_v, w_o):
    q = np.asarray(q, np.float32)
    k = np.asarray(k, np.float32)
    v = np.asarray(v, np.float32)
    w_q = np.asarray(w_q, np.float32)
    w_k = np.asarray(w_k, np.float32)
    w_v = np.asarray(w_v, np.float32)
    w_o = np.asarray(w_o, np.float32)
    maskT = np.ascontiguousarray(
        np.broadcast_to(np.asarray(mask), (1, 1, S, S))[0, 0].T
    )
    cls, bidx, biases = classify_mask(maskT)
    nc = get_program(cls, bidx, len(biases))
    in_maps = make_in_maps(q, k, v, mask, w_q, w_k, w_v, w_o, biases)
    res = run_bass_kernel_spmd(nc, in_maps, list(range(NCORES)))
    return combine_results(res.results)


# revision 9
# speedup vs baseline: 1.0906x; 1.0906x over previous
"""Multi-head attention (B=2, S=2048, D=1024, H=16) on 8 trn2 NeuronCores.

Sharding: batch (2) x head-groups (4 heads each, 4 groups) = 8 cores.
Each core computes Q/K/V projections for its 4 heads on its batch,
causal-masked softmax attention, and a partial output projection
(row-sharded w_o); the host sums the 4 partials per batch.

Layout strategy: the host stages transposed inputs (xT = x[b].T) so every
matmul contraction runs over the SBUF partition axis with no on-device
transposes. Attention scores are computed transposed (ST[k, q]) so that
P = exp(ST) serves directly as the PV matmul's moving operand, and V with
an appended ones-column yields the softmax denominator for free.
The input projections run in bf16; everything downstream runs float32r
(full-rate PE, fp32 storage).
"""
import sys

sys.path.insert(0, "/opt/trn_rl_repo")

import numpy as np
import ml_dtypes

import concourse.bass as bass
import concourse.mybir as mybir
import concourse.tile as tile
from concourse.bass_utils import run_bass_kernel_spmd

B, S, D, H, DK = 2, 2048, 1024, 16, 64
NCORES = 8
HG = 4                # heads per core
DHG = HG * DK         # 256 head-dims per core
KT = D // 128         # 8 contraction tiles for the projections
ST128 = S // 128      # 16 128-row tiles of S
QS = 512              # q-strip width
NQS = S // QS         # 4 strips
NEG = np.float32(-1e9)

f32 = mybir.dt.float32
f32r = mybir.dt.float32r
bf16 = mybir.dt.bfloat16
EXP = mybir.ActivationFunctionType.Exp


def _split_waits(nc, max_waits=1):
    """This walrus build rejects >1 SyncWait per instruction (and >0 on
    fp32-family matmuls, which lower through the 1-wait S3_LW struct).
    Hoist excess waits onto dedicated NOPs on the same engine queue."""
    n = 0
    for fn in nc.m.functions:
        for blk in fn.blocks:
            new = []
            for ins in blk.instructions:
                si = getattr(ins, "sync_info", None)
                if si is not None and si.on_wait:
                    limit = 0 if isinstance(ins, mybir.InstMatmult) else max_waits
                    if len(si.on_wait) > limit:
                        waits = list(si.on_wait)
                        hoist = waits if limit == 0 else waits[:-limit]
                        keep = [] if limit == 0 else waits[-limit:]
                        for w in hoist:
                            n += 1
                            new.append(
                                mybir.InstNoOp(
                                    name=f"I-waitfix-{n}",
                                    engine=ins.engine,
                                    bass_nofuse=True,
                                    sync_info=mybir.SyncInfo(
                                        on_wait=[w], on_update=[]
                                    ),
                                )
                            )
                        ins.sync_info = mybir.SyncInfo(
                            on_wait=keep, on_update=list(si.on_update)
                        )
                new.append(ins)
            blk.instructions[:] = new
    return n


def classify_mask(maskT):
    """Block-classify the transposed mask at 128x128 granularity.
    Returns (cls[i,j] in {0 empty,1 full,2 partial}, bias index map,
    list of additive fp32 bias blocks for the partial ones)."""
    nb = S // 128
    cls = np.empty((nb, nb), dtype=np.int8)
    bidx = np.full((nb, nb), -1, dtype=np.int32)
    biases = []
    for i in range(nb):
        for j in range(nb):
            blk = maskT[i * 128 : (i + 1) * 128, j * 128 : (j + 1) * 128]
            if (blk != 0).all():
                cls[i, j] = 1
            elif (blk == 0).all():
                cls[i, j] = 0
            else:
                cls[i, j] = 2
                bidx[i, j] = len(biases)
                biases.append(
                    np.where(blk == 0, NEG, np.float32(0.0)).astype(np.float32)
                )
    return cls, bidx, biases


def build_program(cls, bidx, n_bias):
    nb_alloc = max(1, n_bias)
    nc = bass.Bass("TRN2", target_bir_lowering=False, debug=False,
                   num_devices=NCORES)
    xq_d = nc.dram_tensor("xqT", [D, S], bf16, kind="ExternalInput").ap()
    xk_d = nc.dram_tensor("xkT", [D, S], bf16, kind="ExternalInput").ap()
    xv_d = nc.dram_tensor("xvT", [D, S], bf16, kind="ExternalInput").ap()
    wq_d = nc.dram_tensor("wqT", [D, DHG], bf16, kind="ExternalInput").ap()
    wk_d = nc.dram_tensor("wkT", [D, DHG], bf16, kind="ExternalInput").ap()
    wv_d = nc.dram_tensor("wvT", [D, DHG], bf16, kind="ExternalInput").ap()
    wo_d = nc.dram_tensor("woT", [DHG, D], f32r, kind="ExternalInput").ap()
    bias_d = nc.dram_tensor("biasT", [nb_alloc, 128, 128], f32,
                            kind="ExternalInput").ap()
    ones_d = nc.dram_tensor("onesr", [1, DK], f32r, kind="ExternalInput").ap()
    y_d = nc.dram_tensor("y", [S, D], f32, kind="ExternalOutput").ap()

    with tile.TileContext(nc) as tc:
        with tc.tile_pool(name="persist", bufs=1) as pp:
            qt_sb = pp.tile([128, 2, S], f32r)            # Q^T, 2 head-pairs
            kt_sb = pp.tile([128, 2, S], f32r)            # K^T
            v_sb = pp.tile([128, ST128, HG, DK + 1], bf16)  # V + ones col
            ot_sb = pp.tile([128, 2, S], f32r)            # attention out^T
            wo_sb = pp.tile([128, 2, D], f32r)
            bias_sb = pp.tile([128, nb_alloc, 128], f32)
            ones_row = pp.tile([1, DK], f32r)
            nc.sync.dma_start(out=ones_row[:], in_=ones_d[:])
            # ones column: memset ALL of v_sb; the V-projection copies
            # overwrite cols 0..DK-1, leaving col DK at 1.0
            nc.vector.memset(
                v_sb[:].rearrange("p a b c -> p (a b c)"), 1.0
            )
            nc.sync.dma_start(
                out=wo_sb[:], in_=wo_d.rearrange("(n p) d -> p n d", p=128)
            )
            if n_bias:
                nc.sync.dma_start(
                    out=bias_sb[:], in_=bias_d.rearrange("n p c -> p n c")
                )

            # ---- Phase A: projections (bf16), strip-split DMAs ----
            with tc.tile_pool(name="xw", bufs=2) as xw, tc.tile_pool(
                name="psA", bufs=4, space="PSUM"
            ) as psA:
                for x_d, w_d, which in (
                    (xq_d, wq_d, "q"),
                    (xk_d, wk_d, "k"),
                    (xv_d, wv_d, "v"),
                ):
                    xt = xw.tile([128, KT, S], bf16, tag="xT")
                    wt = xw.tile([128, KT, DHG], bf16, tag="wT")
                    nc.sync.dma_start(
                        out=wt[:], in_=w_d.rearrange("(n p) s -> p n s", p=128)
                    )
                    xr = x_d.rearrange("(n p) s -> p n s", p=128)
                    for qs in range(NQS):
                        nc.sync.dma_start(
                            out=xt[:, :, qs * QS : (qs + 1) * QS],
                            in_=xr[:, :, qs * QS : (qs + 1) * QS],
                        )
                    if which in ("q", "k"):
                        dst = qt_sb if which == "q" else kt_sb
                        for qs in range(NQS):
                            for mt in range(2):
                                ps = psA.tile([128, QS], f32, tag="pA")
                                for kt in range(KT):
                                    nc.tensor.matmul(
                                        ps[:],
                                        wt[:, kt, mt * 128 : (mt + 1) * 128],
                                        xt[:, kt, qs * QS : (qs + 1) * QS],
                                        start=(kt == 0),
                                        stop=(kt == KT - 1),
                                    )
                                nc.scalar.copy(
                                    out=dst[:, mt, qs * QS : (qs + 1) * QS],
                                    in_=ps[:],
                                )
                    else:
                        for st in range(ST128):
                            ps = psA.tile([128, QS], f32, tag="pA")
                            for kt in range(KT):
                                nc.tensor.matmul(
                                    ps[:, :DHG],
                                    xt[:, kt, st * 128 : (st + 1) * 128],
                                    wt[:, kt, :],
                                    start=(kt == 0),
                                    stop=(kt == KT - 1),
                                )
                            nc.scalar.copy(
                                out=v_sb[:, st, :, 0:DK],
                                in_=ps[:, :DHG].rearrange(
                                    "p (h d) -> p h d", h=HG
                                ),
                            )

            # ---- Phases B+C interleaved per q-strip ----
            # Per strip: attention for all 4 heads (two interleaved pairs so
            # PE streams scores/PV while ACT exps), then the output
            # projection for this strip's rows overlapped into the next.
            with tc.tile_pool(name="pb", bufs=8) as pb, tc.tile_pool(
                name="bc", bufs=4
            ) as bcp, tc.tile_pool(
                name="yp", bufs=3
            ) as yp, tc.tile_pool(
                name="psS", bufs=4, space="PSUM"
            ) as psS, tc.tile_pool(
                name="psOT", bufs=4, space="PSUM"
            ) as psOT:
                for qs in range(NQS):
                    sub_all = cls[:, 4 * qs : 4 * qs + 4]
                    kts = [i for i in range(ST128) if sub_all[i].any()]
                    pots = {}
                    for hpair in range(2):
                        heads = (2 * hpair, 2 * hpair + 1)
                        for h in heads:
                            pots[h] = psOT.tile([DK + 1, QS], f32, tag="pot", name=f"pot{h}")
                        for idx, kt in enumerate(kts):
                            sub = sub_all[kt]
                            nz = np.nonzero(sub)[0]
                            c0 = int(nz.min()) * 128
                            c1 = (int(nz.max()) + 1) * 128
                            pss = {}
                            for h in heads:
                                po = 64 * (h % 2)
                                mt = h // 2
                                ps = psS.tile([128, QS], f32, tag="ps", name=f"ps{h}")
                                pss[h] = ps
                                nc.tensor.matmul(
                                    ps[:, c0:c1],
                                    kt_sb[
                                        po : po + 64, mt,
                                        kt * 128 : (kt + 1) * 128,
                                    ],
                                    qt_sb[
                                        po : po + 64, mt,
                                        qs * QS + c0 : qs * QS + c1,
                                    ],
                                    start=True,
                                    stop=True,
                                )
                            for h in heads:
                                ps = pss[h]
                                for j in range(4):
                                    if sub[j] == 2:
                                        bi = int(bidx[kt, 4 * qs + j])
                                        nc.vector.tensor_add(
                                            ps[:, j * 128 : (j + 1) * 128],
                                            ps[:, j * 128 : (j + 1) * 128],
                                            bias_sb[:, bi, :],
                                        )
                            p_sbs = {}
                            for h in heads:
                                p_sb = pb.tile([128, QS], bf16, tag="p", name=f"p{h}")
                                p_sbs[h] = p_sb
                                if c0 > 0:
                                    nc.vector.memset(p_sb[:, 0:c0], 0.0)
                                if c1 < QS:
                                    nc.vector.memset(p_sb[:, c1:], 0.0)
                                nc.scalar.activation(
                                    p_sb[:, c0:c1], pss[h][:, c0:c1], EXP,
                                    scale=0.125,
                                )
                            for h in heads:
                                nc.tensor.matmul(
                                    pots[h][:],
                                    v_sb[:, kt, h, :],
                                    p_sbs[h][:],
                                    start=(idx == 0),
                                    stop=(idx == len(kts) - 1),
                                )
                    # ---- normalize: per-head reciprocal + PE broadcast ----
                    for h in range(HG):
                        po = 64 * (h % 2)
                        mt = h // 2
                        rec = bcp.tile([1, QS], f32r, tag="rec", name=f"rec{h}")
                        with nc.allow_low_precision(
                            reason="softmax denom reciprocal feeds f32r matmul"
                        ):
                            nc.vector.reciprocal(
                                rec[:], pots[h][DK : DK + 1, :]
                            )
                        pbc = psS.tile([DK, QS], f32, tag="ps", name=f"pbc{h}")
                        nc.tensor.matmul(
                            pbc[:], ones_row[:], rec[:],
                            start=True, stop=True,
                        )
                        bcs = bcp.tile([DK, QS], f32, tag="bc", name=f"bcs{h}")
                        nc.scalar.copy(out=bcs[:], in_=pbc[:])
                        nc.vector.tensor_mul(
                            ot_sb[po : po + 64, mt, qs * QS : (qs + 1) * QS],
                            pots[h][0:DK, :],
                            bcs[:],
                        )
                    # ---- output projection for this strip's rows ----
                    for sti in range(QS // 128):
                        st = qs * (QS // 128) + sti
                        for nh in range(2):
                            ps = psS.tile([128, QS], f32, tag="ps")
                            for mt in range(2):
                                nc.tensor.matmul(
                                    ps[:],
                                    ot_sb[:, mt, st * 128 : (st + 1) * 128],
                                    wo_sb[:, mt, nh * QS : (nh + 1) * QS],
                                    start=(mt == 0),
                                    stop=(mt == 1),
                                )
                            y_sb = yp.tile([128, QS], f32, tag="y")
                            nc.vector.tensor_copy(out=y_sb[:], in_=ps[:])
                            nc.sync.dma_start(
                                out=y_d[
                                    st * 128 : (st + 1) * 128,
                                    nh * QS : (nh + 1) * QS,
                                ],
                                in_=y_sb[:],
                            )

    _split_waits(nc)
    return nc


_program_cache = {}


def get_program(cls, bidx, n_bias):
    key = (cls.tobytes(), bidx.tobytes(), n_bias)
    if key not in _program_cache:
        _program_cache[key] = build_program(cls, bidx, n_bias)
    return _program_cache[key]


def make_in_maps(q, k, v, mask, w_q, w_k, w_v, w_o, biases):
    bias_arr = (
        np.stack(biases).astype(np.float32)
        if biases
        else np.zeros((1, 128, 128), np.float32)
    )
    in_maps = []
    for c in range(NCORES):
        b, g = divmod(c, 4)
        rows = slice(g * DHG, (g + 1) * DHG)
        in_maps.append(
            {
                "xqT": np.ascontiguousarray(q[b].T).astype(ml_dtypes.bfloat16),
                "xkT": np.ascontiguousarray(k[b].T).astype(ml_dtypes.bfloat16),
                "xvT": np.ascontiguousarray(v[b].T).astype(ml_dtypes.bfloat16),
                "wqT": np.ascontiguousarray(w_q[rows].T).astype(
                    ml_dtypes.bfloat16
                ),
                "wkT": np.ascontiguousarray(w_k[rows].T).astype(
                    ml_dtypes.bfloat16
                ),
                "wvT": np.ascontiguousarray(w_v[rows].T).astype(
                    ml_dtypes.bfloat16
                ),
                "woT": np.ascontiguousarray(w_o[:, rows].T).astype(np.float32),
                "biasT": bias_arr,
                "onesr": np.ones((1, DK), np.float32),
            }
        )
    return in_maps


def combine_results(results):
    out = np.empty((B, S, D), np.float32)
    for b in range(B):
        acc = results[4 * b]["y"].astype(np.float32).copy()
        for g in range(1, 4):
            acc += results[4 * b + g]["y"]
        out[b] = acc
    return out


def kernel(q, k, v, mask, w_q, w_k, w_v, w_o):
    q = np.asarray(q, np.float32)
    k = np.asarray(k, np.float32)
    v = np.asarray(v, np.float32)
    w_q = np.asarray(w_q, np.float32)
    w_k = np.asarray(w_k, np.float32)
    w_v = np.asarray(w_v, np.float32)
    w_o = np.asarray(w_o, np.float32)
    maskT = np.ascontiguousarray(
        np.broadcast_to(np.asarray(mask), (1, 1, S, S))[0, 0].T
    )
    cls, bidx, biases = classify_mask(maskT)
    nc = get_program(cls, bidx, len(biases))
    in_maps = make_in_maps(q, k, v, mask, w_q, w_k, w_v, w_o, biases)
    res = run_bass_kernel_spmd(nc, in_maps, list(range(NCORES)))
    return combine_results(res.results)


# revision 10
# speedup vs baseline: 1.1403x; 1.0455x over previous
"""Multi-head attention (B=2, S=2048, D=1024, H=16) on 8 trn2 NeuronCores.

Sharding: batch (2) x head-groups (4 heads each, 4 groups) = 8 cores.
Each core computes Q/K/V projections for its 4 heads on its batch,
causal-masked softmax attention, and a partial output projection
(row-sharded w_o); the host sums the 4 partials per batch.

Layout strategy: the host stages transposed inputs (xT = x[b].T) so every
matmul contraction runs over the SBUF partition axis with no on-device
transposes. Attention scores are computed transposed (ST[k, q]) so that
P = exp(ST) serves directly as the PV matmul's moving operand, and V with
an appended ones-column yields the softmax denominator for free.
The input projections run in bf16; everything downstream runs float32r
(full-rate PE, fp32 storage).
"""
import sys

sys.path.insert(0, "/opt/trn_rl_repo")

import numpy as np
import ml_dtypes

import concourse.bass as bass
import concourse.mybir as mybir
import concourse.tile as tile
from concourse.bass_utils import run_bass_kernel_spmd

B, S, D, H, DK = 2, 2048, 1024, 16, 64
NCORES = 8
HG = 4                # heads per core
DHG = HG * DK         # 256 head-dims per core
KT = D // 128         # 8 contraction tiles for the projections
ST128 = S // 128      # 16 128-row tiles of S
QS = 512              # q-strip width
NQS = S // QS         # 4 strips
NEG = np.float32(-1e9)

f32 = mybir.dt.float32
f32r = mybir.dt.float32r
bf16 = mybir.dt.bfloat16
EXP = mybir.ActivationFunctionType.Exp


def _split_waits(nc, max_waits=1):
    """This walrus build rejects >1 SyncWait per instruction (and >0 on
    fp32-family matmuls, which lower through the 1-wait S3_LW struct).
    Hoist excess waits onto dedicated NOPs on the same engine queue."""
    n = 0
    for fn in nc.m.functions:
        for blk in fn.blocks:
            new = []
            for ins in blk.instructions:
                si = getattr(ins, "sync_info", None)
                if si is not None and si.on_wait:
                    limit = 0 if isinstance(ins, mybir.InstMatmult) else max_waits
                    if len(si.on_wait) > limit:
                        waits = list(si.on_wait)
                        hoist = waits if limit == 0 else waits[:-limit]
                        keep = [] if limit == 0 else waits[-limit:]
                        for w in hoist:
                            n += 1
                            new.append(
                                mybir.InstNoOp(
                                    name=f"I-waitfix-{n}",
                                    engine=ins.engine,
                                    bass_nofuse=True,
                                    sync_info=mybir.SyncInfo(
                                        on_wait=[w], on_update=[]
                                    ),
                                )
                            )
                        ins.sync_info = mybir.SyncInfo(
                            on_wait=keep, on_update=list(si.on_update)
                        )
                new.append(ins)
            blk.instructions[:] = new
    return n


def classify_mask(maskT):
    """Block-classify the transposed mask at 128x128 granularity.
    Returns (cls[i,j] in {0 empty,1 full,2 partial}, bias index map,
    list of additive fp32 bias blocks for the partial ones)."""
    nb = S // 128
    cls = np.empty((nb, nb), dtype=np.int8)
    bidx = np.full((nb, nb), -1, dtype=np.int32)
    biases = []
    for i in range(nb):
        for j in range(nb):
            blk = maskT[i * 128 : (i + 1) * 128, j * 128 : (j + 1) * 128]
            if (blk != 0).all():
                cls[i, j] = 1
            elif (blk == 0).all():
                cls[i, j] = 0
            else:
                cls[i, j] = 2
                bidx[i, j] = len(biases)
                biases.append(
                    np.where(blk == 0, NEG, np.float32(0.0)).astype(np.float32)
                )
    return cls, bidx, biases


def build_program(cls, bidx, n_bias):
    nb_alloc = max(1, n_bias)
    nc = bass.Bass("TRN2", target_bir_lowering=False, debug=False,
                   num_devices=NCORES)
    xq_d = nc.dram_tensor("xqT", [D, S], bf16, kind="ExternalInput").ap()
    xk_d = nc.dram_tensor("xkT", [D, S], bf16, kind="ExternalInput").ap()
    xv_d = nc.dram_tensor("xvT", [D, S], bf16, kind="ExternalInput").ap()
    wq_d = nc.dram_tensor("wqT", [D, DHG], bf16, kind="ExternalInput").ap()
    wk_d = nc.dram_tensor("wkT", [D, DHG], bf16, kind="ExternalInput").ap()
    wv_d = nc.dram_tensor("wvT", [D, DHG], bf16, kind="ExternalInput").ap()
    wo_d = nc.dram_tensor("woT", [DHG, D], f32r, kind="ExternalInput").ap()
    bias_d = nc.dram_tensor("biasT", [nb_alloc, 128, 128], f32r,
                            kind="ExternalInput").ap()
    ident_d = nc.dram_tensor("ident", [128, 128], f32r,
                             kind="ExternalInput").ap()
    y_d = nc.dram_tensor("y", [S, D], f32, kind="ExternalOutput").ap()

    with tile.TileContext(nc) as tc:
        with tc.tile_pool(name="persist", bufs=1) as pp:
            qt_sb = pp.tile([128, 2, S], f32r)            # Q^T, 2 head-pairs
            kt_sb = pp.tile([128, 2, S], f32r)            # K^T
            v_sb = pp.tile([128, ST128, HG, DK + 1], bf16)  # V + ones col
            ot_sb = pp.tile([128, 2, S], f32r)            # attention out^T
            wo_sb = pp.tile([128, 2, D], f32r)
            bias_sb = pp.tile([128, nb_alloc, 128], f32r)
            ident_sb = pp.tile([128, 128], f32r)
            nc.sync.dma_start(out=ident_sb[:], in_=ident_d[:])
            # ones column: memset ALL of v_sb; the V-projection copies
            # overwrite cols 0..DK-1, leaving col DK at 1.0
            nc.vector.memset(
                v_sb[:].rearrange("p a b c -> p (a b c)"), 1.0
            )
            nc.sync.dma_start(
                out=wo_sb[:], in_=wo_d.rearrange("(n p) d -> p n d", p=128)
            )
            if n_bias:
                nc.sync.dma_start(
                    out=bias_sb[:], in_=bias_d.rearrange("n p c -> p n c")
                )

            # ---- Phase A: projections (bf16), strip-split DMAs ----
            with tc.tile_pool(name="xw", bufs=2) as xw, tc.tile_pool(
                name="psA", bufs=4, space="PSUM"
            ) as psA:
                for x_d, w_d, which in (
                    (xq_d, wq_d, "q"),
                    (xk_d, wk_d, "k"),
                    (xv_d, wv_d, "v"),
                ):
                    xt = xw.tile([128, KT, S], bf16, tag="xT")
                    wt = xw.tile([128, KT, DHG], bf16, tag="wT")
                    nc.sync.dma_start(
                        out=wt[:], in_=w_d.rearrange("(n p) s -> p n s", p=128)
                    )
                    xr = x_d.rearrange("(n p) s -> p n s", p=128)
                    for qs in range(NQS):
                        nc.sync.dma_start(
                            out=xt[:, :, qs * QS : (qs + 1) * QS],
                            in_=xr[:, :, qs * QS : (qs + 1) * QS],
                        )
                    if which in ("q", "k"):
                        dst = qt_sb if which == "q" else kt_sb
                        for qs in range(NQS):
                            for mt in range(2):
                                ps = psA.tile([128, QS], f32, tag="pA")
                                for kt in range(KT):
                                    nc.tensor.matmul(
                                        ps[:],
                                        wt[:, kt, mt * 128 : (mt + 1) * 128],
                                        xt[:, kt, qs * QS : (qs + 1) * QS],
                                        start=(kt == 0),
                                        stop=(kt == KT - 1),
                                    )
                                nc.scalar.copy(
                                    out=dst[:, mt, qs * QS : (qs + 1) * QS],
                                    in_=ps[:],
                                )
                    else:
                        for st in range(ST128):
                            ps = psA.tile([128, QS], f32, tag="pA")
                            for kt in range(KT):
                                nc.tensor.matmul(
                                    ps[:, :DHG],
                                    xt[:, kt, st * 128 : (st + 1) * 128],
                                    wt[:, kt, :],
                                    start=(kt == 0),
                                    stop=(kt == KT - 1),
                                )
                            nc.scalar.copy(
                                out=v_sb[:, st, :, 0:DK],
                                in_=ps[:, :DHG].rearrange(
                                    "p (h d) -> p h d", h=HG
                                ),
                            )

            # ---- Phases B+C interleaved per q-strip ----
            # Per strip: attention for all 4 heads (two interleaved pairs so
            # PE streams scores/PV while ACT exps), then the output
            # projection for this strip's rows overlapped into the next.
            with tc.tile_pool(name="pb", bufs=8) as pb, tc.tile_pool(
                name="bc", bufs=4
            ) as bcp, tc.tile_pool(
                name="yp", bufs=3
            ) as yp, tc.tile_pool(
                name="psS", bufs=4, space="PSUM"
            ) as psS, tc.tile_pool(
                name="psOT", bufs=4, space="PSUM"
            ) as psOT:
                for qs in range(NQS):
                    sub_all = cls[:, 4 * qs : 4 * qs + 4]
                    kts = [i for i in range(ST128) if sub_all[i].any()]
                    pots = {}
                    for hpair in range(2):
                        heads = (2 * hpair, 2 * hpair + 1)
                        for h in heads:
                            pots[h] = psOT.tile([DK + 1, QS], f32, tag="pot", name=f"pot{h}")
                        for idx, kt in enumerate(kts):
                            sub = sub_all[kt]
                            nz = np.nonzero(sub)[0]
                            c0 = int(nz.min()) * 128
                            c1 = (int(nz.max()) + 1) * 128
                            partial_js = [
                                j for j in range(4) if sub[j] == 2
                            ]
                            # interior fully-masked blocks (none for causal
                            # or all-ones masks) would need zero-fill
                            interior = [
                                j for j in range(4)
                                if sub[j] == 0 and c0 // 128 < j < c1 // 128
                            ]
                            pss = {}
                            for h in heads:
                                po = 64 * (h % 2)
                                mt = h // 2
                                ps = psS.tile([128, QS], f32, tag="ps", name=f"ps{h}")
                                pss[h] = ps
                                nc.tensor.matmul(
                                    ps[:, c0:c1],
                                    kt_sb[
                                        po : po + 64, mt,
                                        kt * 128 : (kt + 1) * 128,
                                    ],
                                    qt_sb[
                                        po : po + 64, mt,
                                        qs * QS + c0 : qs * QS + c1,
                                    ],
                                    start=True,
                                    stop=(not partial_js),
                                )
                                # fold the mask in on the PE: ps += I.T @ bias
                                for pj, j in enumerate(partial_js):
                                    bi = int(bidx[kt, 4 * qs + j])
                                    nc.tensor.matmul(
                                        ps[:, j * 128 : (j + 1) * 128],
                                        ident_sb[:],
                                        bias_sb[:, bi, :],
                                        start=False,
                                        stop=(pj == len(partial_js) - 1),
                                    )
                            p_sbs = {}
                            for h in heads:
                                p_sb = pb.tile([128, QS], bf16, tag="p", name=f"p{h}")
                                p_sbs[h] = p_sb
                                for j in interior:
                                    nc.vector.memset(
                                        p_sb[:, j * 128 : (j + 1) * 128], 0.0
                                    )
                                nc.scalar.activation(
                                    p_sb[:, c0:c1], pss[h][:, c0:c1], EXP,
                                    scale=0.125,
                                )
                            for h in heads:
                                if idx == 0 and c0 > 0:
                                    nc.vector.memset(pots[h][:, 0:c0], 0.0)
                                nc.tensor.matmul(
                                    pots[h][:, c0:],
                                    v_sb[:, kt, h, :],
                                    p_sbs[h][:, c0:],
                                    start=(idx == 0),
                                    stop=(idx == len(kts) - 1),
                                )
                    # ---- normalize: reciprocal + stream_shuffle broadcast
                    # (DVE only; keeps the PE queue free of stalls) ----
                    for h in range(HG):
                        po = 64 * (h % 2)
                        mt = h // 2
                        rec = bcp.tile([32, QS], f32, tag="rec", name=f"rec{h}")
                        nc.vector.reciprocal(
                            rec[0:1, :], pots[h][DK : DK + 1, :]
                        )
                        bcs = bcp.tile([DK, QS], f32, tag="bc", name=f"bcs{h}")
                        nc.vector.stream_shuffle(
                            out=bcs[0:32, :], in_=rec[:], mask=[0] * 32
                        )
                        nc.vector.stream_shuffle(
                            out=bcs[32:64, :], in_=rec[:], mask=[0] * 32
                        )
                        nc.vector.tensor_mul(
                            ot_sb[po : po + 64, mt, qs * QS : (qs + 1) * QS],
                            pots[h][0:DK, :],
                            bcs[:],
                        )
                    # ---- output projection, pipelined one strip behind
                    # so its matmuls never wait on this strip's normalize ----
                    yproj_strips = [qs - 1] if qs else []
                    if qs == NQS - 1:
                        yproj_strips.append(qs)
                    for yqs in yproj_strips:
                      for sti in range(QS // 128):
                        st = yqs * (QS // 128) + sti
                        for nh in range(2):
                            ps = psS.tile([128, QS], f32, tag="ps")
                            for mt in range(2):
                                nc.tensor.matmul(
                                    ps[:],
                                    ot_sb[:, mt, st * 128 : (st + 1) * 128],
                                    wo_sb[:, mt, nh * QS : (nh + 1) * QS],
                                    start=(mt == 0),
                                    stop=(mt == 1),
                                )
                            y_sb = yp.tile([128, QS], f32, tag="y")
                            nc.vector.tensor_copy(out=y_sb[:], in_=ps[:])
                            nc.sync.dma_start(
                                out=y_d[
                                    st * 128 : (st + 1) * 128,
                                    nh * QS : (nh + 1) * QS,
                                ],
                                in_=y_sb[:],
                            )

    _split_waits(nc)
    return nc


_program_cache = {}


def get_program(cls, bidx, n_bias):
    key = (cls.tobytes(), bidx.tobytes(), n_bias)
    if key not in _program_cache:
        _program_cache[key] = build_program(cls, bidx, n_bias)
    return _program_cache[key]


def make_in_maps(q, k, v, mask, w_q, w_k, w_v, w_o, biases):
    bias_arr = (
        np.stack(biases).astype(np.float32)
        if biases
        else np.zeros((1, 128, 128), np.float32)
    )
    in_maps = []
    for c in range(NCORES):
        b, g = divmod(c, 4)
        rows = slice(g * DHG, (g + 1) * DHG)
        in_maps.append(
            {
                "xqT": np.ascontiguousarray(q[b].T).astype(ml_dtypes.bfloat16),
                "xkT": np.ascontiguousarray(k[b].T).astype(ml_dtypes.bfloat16),
                "xvT": np.ascontiguousarray(v[b].T).astype(ml_dtypes.bfloat16),
                "wqT": np.ascontiguousarray(w_q[rows].T).astype(
                    ml_dtypes.bfloat16
                ),
                "wkT": np.ascontiguousarray(w_k[rows].T).astype(
                    ml_dtypes.bfloat16
                ),
                "wvT": np.ascontiguousarray(w_v[rows].T).astype(
                    ml_dtypes.bfloat16
                ),
                "woT": np.ascontiguousarray(w_o[:, rows].T).astype(np.float32),
                "biasT": bias_arr,
                "ident": np.eye(128, dtype=np.float32),
            }
        )
    return in_maps


def combine_results(results):
    out = np.empty((B, S, D), np.float32)
    for b in range(B):
        acc = results[4 * b]["y"].astype(np.float32).copy()
        for g in range(1, 4):
            acc += results[4 * b + g]["y"]
        out[b] = acc
    return out


def kernel(q, k, v, mask, w_q, w_k, w_v, w_o):
    q = np.asarray(q, np.float32)
    k = np.asarray(k, np.float32)
    v = np.asarray(v, np.float32)
    w_q = np.asarray(w_q, np.float32)
    w_k = np.asarray(w_k, np.float32)
    w_v = np.asarray(w_v, np.float32)
    w_o = np.asarray(w_o, np.float32)
    maskT = np.ascontiguousarray(
        np.broadcast_to(np.asarray(mask), (1, 1, S, S))[0, 0].T
    )
    cls, bidx, biases = classify_mask(maskT)
    nc = get_program(cls, bidx, len(biases))
    in_maps = make_in_maps(q, k, v, mask, w_q, w_k, w_v, w_o, biases)
    res = run_bass_kernel_spmd(nc, in_maps, list(range(NCORES)))
    return combine_results(res.results)
